# revision 5
# baseline (speedup 1.0000x reference)
"""Trainium2 Bass kernel for nn_JointLoss (recon MSE + SimCLR contrastive + group distance loss).

v1 strategy (data-parallel over 8 NeuronCores, SPMD via row-rotated proj):
  - Each core owns a 1024-row block of the 8192x8192 similarity matrix;
    np.roll(proj, -c*1024) puts its own rows at local indices 0..1023.
  - PE: bf16 transposes of P (staged through PSUM-as-bf16) build P^T once;
    bf16 matmuls stream sim chunks [128,2048] into ping-pong PSUM halves.
  - exp + row-sum is SPLIT between two engines working concurrently:
      * ScalarE: spline exp in-place on PSUM with accum_out row-sums.
      * VectorE: Schraudolph fast-exp (affine fp32->int16 bit trick, ~1.8%
        per-element, ~3e-4 per-rowsum error) + 16-bit tensor_scalar accum.
  - GpSimd: fp32->bf16 cast of P, recon-MSE (sub + square-accum), distance
    stats, possum (masked diag sums). No partition reduction on device -
    per-partition partials are shipped and reduced on host in float64.
  - All input DMAs on one HWDGE ring in priority order (proj quarters first,
    then xr/rl) so the sim pipeline starts ~3us in.
"""

import sys

if "/opt/trn_rl_repo" not in sys.path:
    sys.path.insert(0, "/opt/trn_rl_repo")

from contextlib import ExitStack

import numpy as np
import ml_dtypes

import concourse.bacc as bacc
import concourse.tile as tile
from concourse import mybir
from concourse.bass_utils import run_bass_kernel_spmd
from concourse.alu_op_type import AluOpType

N = 8192
D = 128
F = 784
NCORES = 8
RPC = N // NCORES  # 1024 rows per core
RT = RPC // 128    # 8 row-tiles per core
NQ = 4             # column quarters (2048 cols each)
TAU = 0.1

f32 = mybir.dt.float32
bf16 = mybir.dt.bfloat16
fp16 = mybir.dt.float16
i16 = mybir.dt.int16

# Schraudolph fp16 fast-exp: bits = round(x*SCH_A + SCH_B); bits as fp16 ~= exp(x/TAU)
SCH_A = 14773.197218702984   # 1024*log2(e)/TAU
SCH_B = 15302.211261493323   # 15360 + tuned bias (zero-mean rowsum error)

# chunk ownership: ACT if (rt+q)%2==0, plus rt==1 q in (0,2) promoted -> 18/14 split
def _act_own(rt, q):
    return ((rt + q) % 2 == 0) or (rt == 1 and q in (0, 2))


def _kernel_body(tc, proj, xr, rl, identbf, mask, rsum_o, possum_o, partials_o):
    nc = tc.nc
    with ExitStack() as ctx:
        consts = ctx.enter_context(tc.tile_pool(name="consts", bufs=1))
        qf = ctx.enter_context(tc.tile_pool(name="qf", bufs=2))
        qb = ctx.enter_context(tc.tile_pool(name="qb", bufs=2))
        big = ctx.enter_context(tc.tile_pool(name="big", bufs=1))
        dpool = ctx.enter_context(tc.tile_pool(name="dpool", bufs=3))
        stats = ctx.enter_context(tc.tile_pool(name="stats", bufs=1))
        psum = ctx.enter_context(tc.tile_pool(name="psum", bufs=1, space="PSUM"))

        ident_sb = consts.tile([128, 128], bf16)
        nc.sync.dma_start(ident_sb, identbf)
        mask_sb = consts.tile([128, 128], f32)
        nc.sync.dma_start(mask_sb, mask)

        pt_bf = big.tile([128, N], bf16)      # P^T in bf16
        xr_sb = big.tile([128, RT, F], f32)
        rl_sb = big.tile([128, RT, F], f32)
        exp16 = big.tile([128, 2, 2048], i16)  # Schraudolph staging (double buffer)
        junk16 = big.tile([128, 2048], fp16)
        sg2 = big.tile([128, 256, 2], f32)
        sgroups = big.tile([128, 256], f32)

        rsum_sb = stats.tile([128, RT, NQ], f32)
        possum_sb = stats.tile([128, RT], f32)
        partials_sb = stats.tile([128, 4], f32)

        pacc = psum.tile([128, 4096], f32)  # all 8 banks, managed manually

        proj_q = proj.rearrange("(q t p) d -> q p t d", q=NQ, p=128)

        # --- input DMAs: proj quarters FIRST (critical path), then xr/rl ---
        qf_tiles = []
        for q in range(NQ):
            t = qf.tile([128, 16, 128], f32, tag="qf")
            nc.sync.dma_start(t, proj_q[q])
            qf_tiles.append(t)
        nc.sync.dma_start(xr_sb, xr.rearrange("(t p) j -> p t j", p=128))
        nc.sync.dma_start(rl_sb, rl.rearrange("(t p) j -> p t j", p=128))

        # --- GpSimd: fp32->bf16 casts of proj quarters (feeds PE transposes) ---
        qb_tiles = []
        for q in range(NQ):
            t = qb.tile([128, 16, 128], bf16, tag="qb")
            nc.gpsimd.tensor_copy(t, qf_tiles[q])
            qb_tiles.append(t)

        # --- main loop over column quarters ---
        for q in range(NQ):
            # PE transposes of this quarter's 16 tiles -> PSUM cols [0,1024) as bf16
            tslab = pacc[:, 0:1024].bitcast(bf16)  # [128, 2048] bf16
            for tl in range(16):
                nc.tensor.transpose(
                    tslab[:, tl * 128 : (tl + 1) * 128], qb_tiles[q][:, tl, :], ident_sb
                )
            # DVE copies the transposed slab out to pt_bf
            nc.vector.tensor_copy(pt_bf[:, q * 2048 : (q + 1) * 2048], tslab)

            for rt in range(RT):
                half = rt % 2
                chunk = pacc[:, half * 2048 : half * 2048 + 2048]
                w = pt_bf[:, rt * 128 : (rt + 1) * 128]
                for c in range(4):
                    nc.tensor.matmul(
                        chunk[:, c * 512 : (c + 1) * 512],
                        w,
                        pt_bf[:, q * 2048 + c * 512 : q * 2048 + (c + 1) * 512],
                        start=True,
                        stop=True,
                    )
                if q == 0:
                    # exact exp of the positive (diagonal) block -> possum path
                    diag_sb = dpool.tile([128, 128], f32, tag="diag")
                    nc.scalar.activation(
                        diag_sb,
                        chunk[:, rt * 128 : rt * 128 + 128],
                        mybir.ActivationFunctionType.Exp,
                        scale=1.0 / TAU,
                    )
                    dm = dpool.tile([128, 128], bf16, tag="dm")
                    nc.gpsimd.tensor_tensor(dm, diag_sb, mask_sb, AluOpType.mult)
                    nc.vector.tensor_scalar(
                        dm, dm, 1.0, 0.0, AluOpType.mult, AluOpType.add,
                        accum_out=possum_sb[:, rt : rt + 1],
                    )
                if _act_own(rt, q):
                    # ScalarE: spline exp in-place + accumulated row-sum
                    nc.scalar.activation(
                        chunk,
                        chunk,
                        mybir.ActivationFunctionType.Exp,
                        scale=1.0 / TAU,
                        accum_out=rsum_sb[:, rt, q : q + 1],
                    )
                else:
                    # VectorE: Schraudolph affine -> int16, then 16-bit accum pass
                    st = exp16[:, q % 2, :]
                    nc.vector.tensor_scalar(
                        st, chunk, SCH_A, SCH_B, AluOpType.mult, AluOpType.add
                    )
                    nc.vector.tensor_scalar(
                        junk16,
                        st.bitcast(fp16),
                        1.0,
                        0.0,
                        AluOpType.mult,
                        AluOpType.add,
                        accum_out=rsum_sb[:, rt, q : q + 1],
                    )

        # --- GpSimd: recon MSE and distance-loss stats (after xr/rl arrive) ---
        # GpSimd does the elementwise work (bf16 out); DVE accumulates at 4x.
        diffb = big.tile([128, RT, F], bf16)
        nc.gpsimd.tensor_tensor(diffb, xr_sb, rl_sb, AluOpType.subtract)
        nc.gpsimd.tensor_tensor(diffb, diffb, diffb, AluOpType.mult)
        nc.vector.tensor_scalar(
            diffb, diffb, 1.0, 0.0, AluOpType.mult, AluOpType.add,
            accum_out=partials_sb[:, 0:1],
        )
        pt4 = pt_bf[:, 0:RPC].rearrange("p (g s) -> p g s", s=4)
        nc.gpsimd.tensor_tensor(sg2, pt4[:, :, 0::2], pt4[:, :, 1::2], AluOpType.add)
        nc.gpsimd.tensor_tensor(sgroups, sg2[:, :, 0], sg2[:, :, 1], AluOpType.add)
        nc.gpsimd.tensor_tensor(sgroups, sgroups, sgroups, AluOpType.mult)
        nc.vector.tensor_scalar(
            sgroups, sgroups, 1.0, 0.0, AluOpType.mult, AluOpType.add,
            accum_out=partials_sb[:, 2:3],
        )
        pown = pt_bf[:, 0:RPC]
        nc.gpsimd.tensor_tensor(pown, pown, pown, AluOpType.mult)
        nc.vector.tensor_scalar(
            pown, pown, 1.0, 0.0, AluOpType.mult, AluOpType.add,
            accum_out=partials_sb[:, 1:2],
        )
        nc.gpsimd.memset(partials_sb[:, 3:4], 0.0)

        nc.sync.dma_start(rsum_o, rsum_sb.rearrange("p t q -> p (t q)"))
        nc.sync.dma_start(possum_o, possum_sb)
        nc.sync.dma_start(partials_o, partials_sb)


def _build():
    nc = bacc.Bacc("TRN2", target_bir_lowering=False, debug=False, num_devices=NCORES)
    proj = nc.dram_tensor("proj", [N, D], f32, kind="ExternalInput").ap()
    xr = nc.dram_tensor("xr", [RPC, F], f32, kind="ExternalInput").ap()
    rl = nc.dram_tensor("rl", [RPC, F], f32, kind="ExternalInput").ap()
    identbf = nc.dram_tensor("identbf", [128, 128], bf16, kind="ExternalInput").ap()
    mask = nc.dram_tensor("mask", [128, 128], f32, kind="ExternalInput").ap()
    rsum_o = nc.dram_tensor("rsum_o", [128, RT * NQ], f32, kind="ExternalOutput").ap()
    possum_o = nc.dram_tensor("possum_o", [128, RT], f32, kind="ExternalOutput").ap()
    partials_o = nc.dram_tensor("partials_o", [128, 4], f32, kind="ExternalOutput").ap()

    with tile.TileContext(nc) as tc:
        _kernel_body(tc, proj, xr, rl, identbf, mask, rsum_o, possum_o, partials_o)
    nc.compile()
    return nc


_NC_CACHE = None


def _get_nc():
    global _NC_CACHE
    if _NC_CACHE is None:
        _NC_CACHE = _build()
    return _NC_CACHE


def _run(projections, xrecon, recon_label, trace=False, **spmd_kwargs):
    nc = _get_nc()
    P = np.ascontiguousarray(np.asarray(projections, dtype=np.float32))
    XR = np.ascontiguousarray(np.asarray(xrecon, dtype=np.float32))
    RL = np.ascontiguousarray(np.asarray(recon_label, dtype=np.float32))
    identbf = np.eye(128, dtype=ml_dtypes.bfloat16)
    mask = np.kron(np.eye(32, dtype=np.float32), np.ones((4, 4), dtype=np.float32))
    in_maps = []
    for c in range(NCORES):
        in_maps.append(
            {
                "proj": np.ascontiguousarray(np.roll(P, -c * RPC, axis=0)),
                "xr": np.ascontiguousarray(XR[c * RPC : (c + 1) * RPC]),
                "rl": np.ascontiguousarray(RL[c * RPC : (c + 1) * RPC]),
                "identbf": identbf,
                "mask": mask,
            }
        )
    return run_bass_kernel_spmd(
        nc, in_maps, core_ids=list(range(NCORES)), trace=trace, **spmd_kwargs
    )


def _combine(results):
    rowsum = np.concatenate(
        [
            results[c]["rsum_o"].reshape(128, RT, NQ).sum(-1).T.reshape(-1)
            for c in range(NCORES)
        ]
    ).astype(np.float64)
    possum = np.concatenate(
        [results[c]["possum_o"].T.reshape(-1) for c in range(NCORES)]
    ).astype(np.float64)
    recon_ss = sum(
        float(results[c]["partials_o"][:, 0].astype(np.float64).sum())
        for c in range(NCORES)
    )
    A = sum(
        float(results[c]["partials_o"][:, 1].astype(np.float64).sum())
        for c in range(NCORES)
    )
    B = sum(
        float(results[c]["partials_o"][:, 2].astype(np.float64).sum())
        for c in range(NCORES)
    )
    closs = float(np.mean(np.log(rowsum) - np.log(possum)))
    recon_loss = recon_ss / (N * F)
    dist_loss = (4.0 * A - B) / ((N // 4) * 6 * D)
    loss = closs + recon_loss + dist_loss
    return (
        np.float32(loss),
        np.float32(closs),
        np.float32(recon_loss),
        np.float32(dist_loss),
    )


def kernel(projections, xrecon, recon_label):
    br = _run(projections, xrecon, recon_label)
    return _combine(br.results)
